# revision 20
# baseline (speedup 1.0000x reference)
"""Trainium2 Bass kernel for the nn_Dynamics problem.

Math (per batch element, d=8, H=128):
  x = X[:, :8], v = X[:, 8:]
  z0 = W0 x + b0; h0 = tanh(z0); z1 = W1 h0 + b1; h1 = tanh(z1)
  a1 = (1-h1^2)*w2;  A0 = W1^T a1;  a0 = (1-h0^2)*A0;  g = W0^T a0
  t0 = W0 v; t1 = W1((1-h0^2) t0)
  hvv = -2 sum_k [a1 h1 t1^2 + A0 h0 (1-h0^2) t0^2]
  force = -(K x + D v)
  out = force - g*(g.force + hvv)/(1 + |g|^2)      (Sherman-Morrison)

Sign convention (primed = negated): m0 = h0^2-1
  h0p' = m0 t0; t1' = -t1; u' = h0 m0 t0^2; a0' = m0 A0; gn = -g
  e1 = a1 h1 t1'^2;  e2' = A0 u';  ecomb = e2' - e1;  hvv = 2 sum(ecomb)
  den = 1+|gn|^2; gp = sum(gn*f); num = hv - gp; out = f + gn*num/den

Layout: features on partitions, batch on the free axis, tiles of 512.
The input transpose is done on the HOST (X.T is passed both as f16 for
the MLP matmuls and f32 for the force path).  The force computation is
folded into the batch-major output transposes: the moving operand of the
transpose matmul is an augmented map M s.t. out = [(-K^T x - D^T v), gn, hv]
lands batch-major directly.  The per-element scalar tail runs batch-major,
batched over 4 tiles.  a1 and e2 run on GpSimd; psum->sbuf staging is DMA.

Sharding: pure data parallel over 8 NeuronCores (8192 rows each), weights
replicated, outputs concatenated (host concat, rows already in order).
"""

import os

import numpy as np

import concourse.bacc as bacc
import concourse.bass as bass
import concourse.dve_ops as dve_ops
import concourse.tile as tile
from concourse import mybir
from concourse.bass_utils import run_bass_kernel_spmd
from concourse.dve_ops import DveOp
from concourse.dve_ops import has_src1
from concourse.dve_spec import C0, C2, One, Spec, Src0, Src1, lower, sq
from concourse.dve_uop import DveOpSpec

F32 = mybir.dt.float32
F32R = mybir.dt.float32r
F16 = mybir.dt.float16
AX = mybir.AxisListType
OP = mybir.AluOpType
ACT = mybir.ActivationFunctionType

DIM = 8
H = 128
BATCH = 65536
NCORES = 8
BC = BATCH // NCORES          # 8192 rows per core
TW = 512                      # batch tile width
NT = BC // TW                 # 16 tiles per core
NCH = TW // 128               # 4 chunks of 128 per tile
NBT = 4                       # tiles per tail batch
NG = NT // NBT                # tail groups
CW = 2 * DIM + 1              # 17 packed batch-major cols per chunk

LAST_RESULTS = None

# ---------------- custom fused DVE ops ----------------


def _register_op(name, body, reference):
    if name in dve_ops._SUB_OPCODE_FOR_NAME:
        for op in dve_ops.OPS:
            if op.name == name:
                return op
    spec = Spec(body=body, reference=reference)
    shas = {}
    for ver in ("v3", "v4"):
        shas[ver] = DveOpSpec(
            name=name,
            opcode=dve_ops._CUSTOM_DVE_ROW_BASE + len(dve_ops.OPS),
            uops=lower(spec, ver=ver),
            rd1_en=has_src1(spec),
        ).sha(ver)
    op = DveOp(name, spec, subdim=False, uops_sha=shas)
    dve_ops.OPS.append(op)
    dve_ops.CUSTOM_DVE_SPECS[name] = spec
    dve_ops._SUB_OPCODE_FOR_NAME[name] = (
        dve_ops._CUSTOM_DVE_ROW_BASE + len(dve_ops.OPS) - 1
    )
    return op


# h0p' = (h0^2 - 1) * t0     (also a0' = (h0^2 - 1) * A0)
OP_SQM1_MUL = _register_op(
    "ANT_SQM1_MUL",
    (sq(Src0) - One) * Src1,
    lambda in0, in1: (in0 * in0 - 1.0) * in1,
)
# u' = h0 * (h0^2 - 1) * t0^2
OP_UPRIME = _register_op(
    "ANT_UPRIME",
    Src0 * (sq(Src0) - One) * sq(Src1),
    lambda in0, in1: in0 * (in0 * in0 - 1.0) * in1 * in1,
)
# e1 = (1 - h1^2) * w2 * h1 * t1^2
OP_E1F = _register_op(
    "ANT_E1F",
    (One - sq(Src0)) * C0 * Src0 * sq(Src1),
    lambda in0, in1, s0: (1.0 - in0 * in0) * s0 * in0 * in1 * in1,
)
# gsq with 1/8 folded in: sum over 8 features gives 1+|g|^2 directly
OP_SQP = _register_op(
    "ANT_SQP",
    sq(Src0) + C2,
    lambda in0, imm2: in0 * in0 + imm2,
)


def build_nc():
    nc = bacc.Bacc()

    XT16 = nc.dram_tensor("XT16", [2 * DIM, BC], F16, kind="ExternalInput")
    XT32 = nc.dram_tensor("XT32", [2 * DIM, BC], F32, kind="ExternalInput")
    W0Tx = nc.dram_tensor("W0Tx", [2 * DIM, H], F16, kind="ExternalInput")
    W0Tv = nc.dram_tensor("W0Tv", [2 * DIM, H], F16, kind="ExternalInput")
    W1T = nc.dram_tensor("W1T", [H, H], F16, kind="ExternalInput")
    W1 = nc.dram_tensor("W1", [H, H], F16, kind="ExternalInput")
    W016 = nc.dram_tensor("W016", [H, 16], F16, kind="ExternalInput")
    P216 = nc.dram_tensor("P216", [H, 16], F16, kind="ExternalInput")
    MMAP = nc.dram_tensor("MMAP", [32, CW], F32, kind="ExternalInput")
    b0c = nc.dram_tensor("b0c", [H, 1], F32, kind="ExternalInput")
    b1c = nc.dram_tensor("b1c", [H, 1], F32, kind="ExternalInput")
    w2c = nc.dram_tensor("w2c", [H, 1], F32, kind="ExternalInput")
    w2n = nc.dram_tensor("w2n", [H, 1], F32, kind="ExternalInput")
    out = nc.dram_tensor("out", [BC, DIM], F32, kind="ExternalOutput")

    from contextlib import ExitStack

    with tile.TileContext(nc) as tc, ExitStack() as stk:
        consts = stk.enter_context(tc.tile_pool(name="consts", bufs=1))
        work = stk.enter_context(tc.tile_pool(name="work", bufs=2))
        tbp = stk.enter_context(tc.tile_pool(name="tbp", bufs=2))
        ps_zz = stk.enter_context(tc.tile_pool(name="ps_zz", bufs=2, space="PSUM"))
        ps_tt = stk.enter_context(tc.tile_pool(name="ps_tt", bufs=2, space="PSUM"))
        ps_aa = stk.enter_context(tc.tile_pool(name="ps_aa", bufs=2, space="PSUM"))
        ps_fm = stk.enter_context(tc.tile_pool(name="ps_fm", bufs=1, space="PSUM"))
        ps_bm = stk.enter_context(tc.tile_pool(name="ps_bm", bufs=1, space="PSUM"))

        # ---------------- constants ----------------
        XT_sb = consts.tile([2 * DIM, BC], F16)
        nc.sync.dma_start(out=XT_sb, in_=XT16[:, :])
        # E_all rows: 0:8 = gn (per tile), 8 = hv, 16:32 = X^T (f32)
        E_all = consts.tile([32, BC], F32)
        nc.sync.dma_start(out=E_all[16:32, :], in_=XT32[:, :])

        W0Tx_sb = consts.tile([2 * DIM, H], F16)
        nc.sync.dma_start(out=W0Tx_sb, in_=W0Tx[:, :])
        W0Tv_sb = consts.tile([2 * DIM, H], F16)
        nc.sync.dma_start(out=W0Tv_sb, in_=W0Tv[:, :])
        W1T_sb = consts.tile([H, H], F16)
        nc.sync.dma_start(out=W1T_sb, in_=W1T[:, :])
        W1_sb = consts.tile([H, H], F16)
        nc.sync.dma_start(out=W1_sb, in_=W1[:, :])
        W016_sb = consts.tile([H, 16], F16)
        nc.sync.dma_start(out=W016_sb, in_=W016[:, :])
        P216_sb = consts.tile([H, 16], F16)
        nc.sync.dma_start(out=P216_sb, in_=P216[:, :])
        M_sb = consts.tile([32, CW], F32)
        nc.sync.dma_start(out=M_sb, in_=MMAP[:, :])
        b0_sb = consts.tile([H, 1], F32)
        nc.sync.dma_start(out=b0_sb, in_=b0c[:, :])
        b1_sb = consts.tile([H, 1], F32)
        nc.sync.dma_start(out=b1_sb, in_=b1c[:, :])
        w2_sb = consts.tile([H, 1], F32)
        nc.sync.dma_start(out=w2_sb, in_=w2c[:, :])
        w2n_sb = consts.tile([H, 1], F32)
        nc.sync.dma_start(out=w2n_sb, in_=w2n[:, :])

        ones_sb = consts.tile([H, TW], F16)
        nc.vector.memset(ones_sb, 1.0)

        out_sb = consts.tile([128, (BC // 128) * DIM], F32)

        E_r = E_all
        M_r = M_sb

        # ---------------- main loop ----------------
        for t in range(NT):
            sl = slice(TW * t, TW * (t + 1))

            z0 = ps_zz.tile([H, TW], F32, tag="zz")
            nc.tensor.matmul(z0, W0Tx_sb, XT_sb[:, sl], start=True, stop=True)
            t0 = ps_tt.tile([H, TW], F32, tag="tt")
            nc.tensor.matmul(t0, W0Tv_sb, XT_sb[:, sl], start=True, stop=True)

            h0 = work.tile([H, TW], F16, tag="h0")
            nc.scalar.activation(h0, z0, ACT.Tanh, bias=b0_sb, scale=1.0)

            h0p = work.tile([H, TW], F16, tag="h0p")
            nc.vector._custom_dve(OP_SQM1_MUL, out=h0p, in0=h0, in1=t0[:, :])
            u = work.tile([H, TW], F16, tag="u")
            nc.vector._custom_dve(OP_UPRIME, out=u, in0=h0, in1=t0[:, :])

            z1 = ps_zz.tile([H, TW], F32, tag="zz")
            nc.tensor.matmul(z1, W1T_sb, h0, start=True, stop=True)
            t1 = ps_tt.tile([H, TW], F32, tag="tt")
            nc.tensor.matmul(t1, W1T_sb, h0p, start=True, stop=True)

            h1 = work.tile([H, TW], F16, tag="h1")
            nc.scalar.activation(h1, z1, ACT.Tanh, bias=b1_sb, scale=1.0)

            # d1m = h1^2 - 1 = -(1-h1^2); w2 is folded into the A0 stationary
            # (W1w[k,m] = W1[k,m]*w2[k]) so A0n = -W1^T a1.
            h1sq = work.tile([H, TW], F16, tag="h1sq")
            nc.vector.tensor_mul(h1sq, h1, h1)
            d1m = work.tile([H, TW], F16, tag="d1m")
            nc.gpsimd.tensor_sub(d1m, h1sq, ones_sb)

            # e1n = -e1  (w2 negated via s0)
            e1 = work.tile([H, TW], F16, tag="e1")
            nc.vector._custom_dve(
                OP_E1F, out=e1, in0=h1, in1=t1[:, :], s0=w2n_sb[:, 0:1]
            )

            A0 = ps_aa.tile([H, TW], F32, tag="aa")
            nc.tensor.matmul(A0, W1_sb, d1m, start=True, stop=True)

            # A0 = -W1^T a1: a0 = (h0^2-1)*A0n = +(1-h0^2) W1^T a1 (true sign)
            a0 = work.tile([H, TW], F16, tag="a0")
            nc.vector._custom_dve(OP_SQM1_MUL, out=a0, in0=h0, in1=A0[:, :])
            e2 = work.tile([H, TW], F16, tag="e2")
            nc.vector.tensor_mul(e2, A0[:, :], u)
            # ecomb = e1n - e2 = -e1 - e2 on GpSimd (plain TT only)
            ecomb = work.tile([H, TW], F16, tag="ec")
            nc.gpsimd.tensor_sub(ecomb, e1, e2)

            # feature-major mini-block: hvv rows 8:16 (start), gn rows 0:8
            # accumulated on top (W016 cols 8:16 are zero, P216 cols 0:8 zero)
            fm = ps_fm.tile([16, TW], F32, tag="fm")
            nc.tensor.matmul(fm, P216_sb, ecomb, start=True, stop=False)
            nc.tensor.matmul(fm, W016_sb, a0, start=False, stop=True)

            # stage gn/hv rows next to X^T rows for the fused transpose
            nc.scalar.copy(E_all[0:16, sl], fm)

            # fused transpose + force map: bm[:, 17c+q] per 128-chunk
            bm = ps_bm.tile([128, NCH * CW], F32, tag="bm")
            for c in range(NCH):
                nc.tensor.matmul(
                    bm[:, CW * c : CW * (c + 1)],
                    E_r[:, TW * t + 128 * c : TW * t + 128 * (c + 1)],
                    M_r,
                    start=True,
                    stop=True,
                )

            g = t // NBT
            dt = t % NBT
            if dt == 0:
                tb4 = tbp.tile([128, NBT * NCH * CW], F32, tag="tb")
            nc.scalar.copy(tb4[:, NCH * CW * dt : NCH * CW * (dt + 1)], bm)

            if dt == NBT - 1:
                # ---- batched batch-major tail over NBT tiles ----
                B = NBT * NCH  # 16 chunks
                def col3(off, w):
                    return bass.AP(
                        tensor=tb4.tensor,
                        offset=tb4.offset + off,
                        ap=[list(tb4.ap[0]), [CW, B], [1, w]],
                    )

                f3 = col3(0, DIM)
                gn3 = col3(DIM, DIM)
                hv2 = bass.AP(
                    tensor=tb4.tensor,
                    offset=tb4.offset + 2 * DIM,
                    ap=[list(tb4.ap[0]), [CW, B]],
                )

                gb = tbp.tile([128, 2 * B * DIM], F32, tag="gb")
                gb4 = gb.rearrange("p (q j f) -> p q j f", q=2, f=DIM)
                nc.vector._custom_dve(
                    OP_SQP, out=gb4[:, 0], in0=gn3, imm2=1.0 / DIM
                )
                nc.gpsimd.tensor_mul(gb4[:, 1], gn3, f3)
                red = tbp.tile([128, 2 * B], F32, tag="red")
                red3 = red.rearrange("p (q j) -> p q j", q=2)
                nc.vector.tensor_reduce(red3, gb4, axis=AX.X, op=OP.add)
                num = tbp.tile([128, B], F32, tag="num")
                nc.gpsimd.tensor_sub(num, hv2, red3[:, 1])
                rec = tbp.tile([128, B], F32, tag="rec")
                nc.vector.reciprocal(rec, red3[:, 0])
                s = tbp.tile([128, B], F32, tag="s")
                nc.gpsimd.tensor_mul(s, num, rec)
                sbc = bass.AP(
                    tensor=s.tensor,
                    offset=s.offset,
                    ap=[list(s.ap[0]), [1, B], [0, DIM]],
                )
                su = tbp.tile([128, B * DIM], F32, tag="su")
                su3 = su.rearrange("p (j f) -> p j f", f=DIM)
                nc.gpsimd.tensor_mul(su3, gn3, sbc)
                ob = out_sb[:, B * DIM * g : B * DIM * (g + 1)]
                nc.gpsimd.tensor_add(
                    ob.rearrange("p (j f) -> p j f", f=DIM), f3, su3
                )

        nc.sync.dma_start(
            out=out.rearrange("(j p) f -> p j f", p=128),
            in_=out_sb.rearrange("p (j f) -> p j f", f=DIM),
        )

    if not nc.is_finalized():
        nc.finalize()

    return nc


_NC_CACHE = None


def _install_ntff_shim():
    """Register the axon NTFF profile hook (missing antenv.axon_hooks shim)."""
    import sys
    import types

    if "antenv.axon_hooks" in sys.modules:
        return
    try:
        sys.path.insert(0, "/root/.axon_site")
        from trn_agent_boot.trn_boot import _ntff_profile_via_ctypes

        hook = _ntff_profile_via_ctypes("/opt/axon/libaxon_pjrt.so")
        mod = types.ModuleType("antenv.axon_hooks")
        mod.get_axon_ntff_profile_hook = lambda: hook
        sys.modules["antenv.axon_hooks"] = mod
    except Exception:
        pass


def kernel(**inputs):
    global LAST_RESULTS, _NC_CACHE
    trace = bool(int(os.environ.get("KERNEL_TRACE", "0")))
    if trace:
        _install_ntff_shim()
    if _NC_CACHE is None:
        _NC_CACHE = build_nc()
    nc = _NC_CACHE

    X = np.ascontiguousarray(inputs["X"], dtype=np.float32)
    K = np.asarray(inputs["K"], np.float32)
    D = np.asarray(inputs["D"], np.float32)
    W0 = np.asarray(inputs["W0"], np.float32)
    W1 = np.asarray(inputs["W1"], np.float32)
    W2 = np.asarray(inputs["W2"], np.float32)

    w0tx = np.zeros((2 * DIM, H), np.float32)
    w0tx[0:DIM] = W0.T
    w0tv = np.zeros((2 * DIM, H), np.float32)
    w0tv[DIM:] = W0.T
    # M: col q<8: force map (-K^T on x rows 16:24, -D^T on v rows 24:32);
    # cols 8:16 pass gn rows 0:8; col 16 passes hv row 8.
    M = np.zeros((32, CW), np.float32)
    M[16:24, 0:DIM] = -K.T
    M[24:32, 0:DIM] = -D.T
    for i in range(DIM):
        M[i, DIM + i] = 1.0
    M[DIM, 2 * DIM] = -1.0  # hv row negated: num = -hv - g.f

    w016 = np.zeros((H, 16), np.float32)
    w016[:, 0:DIM] = W0
    p216 = np.zeros((H, 16), np.float32)
    p216[:, DIM:] = 2.0

    shared = {
        "W0Tx": w0tx.astype(np.float16),
        "W0Tv": w0tv.astype(np.float16),
        "W1T": np.ascontiguousarray(W1.T).astype(np.float16),
        "W1": (W1 * W2.reshape(H, 1)).astype(np.float16),
        "W016": w016.astype(np.float16),
        "P216": p216.astype(np.float16),
        "MMAP": M,
        "b0c": np.asarray(inputs["b0"], np.float32).reshape(H, 1).copy(),
        "b1c": np.asarray(inputs["b1"], np.float32).reshape(H, 1).copy(),
        "w2c": W2.reshape(H, 1).copy(),
        "w2n": (-W2).reshape(H, 1).copy(),
    }
    in_maps = []
    for i in range(NCORES):
        xt = np.ascontiguousarray(X[i * BC : (i + 1) * BC].T)
        m = {"XT16": xt.astype(np.float16), "XT32": xt}
        m.update(shared)
        in_maps.append(m)

    res = run_bass_kernel_spmd(
        nc, in_maps, core_ids=list(range(NCORES)), trace=trace
    )
    LAST_RESULTS = res
    out_full = np.concatenate([res.results[i]["out"] for i in range(NCORES)], axis=0)
    return out_full.astype(np.float32)


# revision 27
# speedup vs baseline: 1.2032x; 1.2032x over previous
"""Trainium2 Bass kernel for the nn_Dynamics problem.

Math (per batch element, d=8, H=128):
  x = X[:, :8], v = X[:, 8:]
  z0 = W0 x + b0; h0 = tanh(z0); z1 = W1 h0 + b1; h1 = tanh(z1)
  a1 = (1-h1^2)*w2;  A0 = W1^T a1;  a0 = (1-h0^2)*A0;  g = W0^T a0
  t0 = W0 v; t1 = W1((1-h0^2) t0)
  hvv = -2 sum_k [a1 h1 t1^2 + A0 h0 (1-h0^2) t0^2]
  force = -(K x + D v)
  out = force - g*(g.force + hvv)/(1 + |g|^2)      (Sherman-Morrison)

Sign convention (primed = negated): m0 = h0^2-1
  h0p' = m0 t0; t1' = -t1; u' = h0 m0 t0^2; a0' = m0 A0; gn = -g
  e1 = a1 h1 t1'^2;  e2' = A0 u';  ecomb = e2' - e1;  hvv = 2 sum(ecomb)
  den = 1+|gn|^2; gp = sum(gn*f); num = hv - gp; out = f + gn*num/den

Layout: features on partitions, batch on the free axis, tiles of 512.
The input transpose is done on the HOST (X.T is passed both as f16 for
the MLP matmuls and f32 for the force path).  The force computation is
folded into the batch-major output transposes: the moving operand of the
transpose matmul is an augmented map M s.t. out = [(-K^T x - D^T v), gn, hv]
lands batch-major directly.  The per-element scalar tail runs batch-major,
batched over 4 tiles.  a1 and e2 run on GpSimd; psum->sbuf staging is DMA.

Sharding: pure data parallel over 8 NeuronCores (8192 rows each), weights
replicated, outputs concatenated (host concat, rows already in order).
"""

import os

import numpy as np

import concourse.bacc as bacc
import concourse.bass as bass
import concourse.dve_ops as dve_ops
import concourse.tile as tile
from concourse import mybir
from concourse.bass_utils import run_bass_kernel_spmd
from concourse.dve_ops import DveOp
from concourse.dve_ops import has_src1
from concourse.dve_spec import C0, C2, One, Spec, Src0, Src1, lower, sq
from concourse.dve_uop import DveOpSpec

F32 = mybir.dt.float32
F32R = mybir.dt.float32r
F16 = mybir.dt.float16
AX = mybir.AxisListType
OP = mybir.AluOpType
ACT = mybir.ActivationFunctionType

DIM = 8
H = 128
BATCH = 65536
NCORES = 8
BC = BATCH // NCORES          # 8192 rows per core
TW = 512                      # batch tile width
NT = BC // TW                 # 16 tiles per core
NCH = TW // 128               # 4 chunks of 128 per tile
NBT = 4                       # tiles per tail batch
NG = NT // NBT                # tail groups
CW = 2 * DIM + 1              # 17 packed batch-major cols per chunk

LAST_RESULTS = None

# ---------------- custom fused DVE ops ----------------


def _register_op(name, body, reference):
    if name in dve_ops._SUB_OPCODE_FOR_NAME:
        for op in dve_ops.OPS:
            if op.name == name:
                return op
    spec = Spec(body=body, reference=reference)
    shas = {}
    for ver in ("v3", "v4"):
        shas[ver] = DveOpSpec(
            name=name,
            opcode=dve_ops._CUSTOM_DVE_ROW_BASE + len(dve_ops.OPS),
            uops=lower(spec, ver=ver),
            rd1_en=has_src1(spec),
        ).sha(ver)
    op = DveOp(name, spec, subdim=False, uops_sha=shas)
    dve_ops.OPS.append(op)
    dve_ops.CUSTOM_DVE_SPECS[name] = spec
    dve_ops._SUB_OPCODE_FOR_NAME[name] = (
        dve_ops._CUSTOM_DVE_ROW_BASE + len(dve_ops.OPS) - 1
    )
    return op


# h0p' = (h0^2 - 1) * t0     (also a0' = (h0^2 - 1) * A0)
OP_SQM1_MUL = _register_op(
    "ANT_SQM1_MUL",
    (sq(Src0) - One) * Src1,
    lambda in0, in1: (in0 * in0 - 1.0) * in1,
)
# u' = h0 * (h0^2 - 1) * t0^2
OP_UPRIME = _register_op(
    "ANT_UPRIME",
    Src0 * (sq(Src0) - One) * sq(Src1),
    lambda in0, in1: in0 * (in0 * in0 - 1.0) * in1 * in1,
)
# e1 = (1 - h1^2) * w2 * h1 * t1^2
OP_E1F = _register_op(
    "ANT_E1F",
    (One - sq(Src0)) * C0 * Src0 * sq(Src1),
    lambda in0, in1, s0: (1.0 - in0 * in0) * s0 * in0 * in1 * in1,
)
# gsq with 1/8 folded in: sum over 8 features gives 1+|g|^2 directly
OP_SQP = _register_op(
    "ANT_SQP",
    sq(Src0) + C2,
    lambda in0, imm2: in0 * in0 + imm2,
)
# a0 = (h0^2-1) * (A0raw - c0)   (c0 = column sum of W1w, folded -1)
OP_SQM1_MULS = _register_op(
    "ANT_SQM1_MULS",
    (sq(Src0) - One) * (Src1 - C0),
    lambda in0, in1, s0: (in0 * in0 - 1.0) * (in1 - s0),
)
# e2 = (A0raw - c0) * u
OP_MULS = _register_op(
    "ANT_MULS",
    (Src0 - C0) * Src1,
    lambda in0, in1, s0: (in0 - s0) * in1,
)


def build_nc():
    nc = bacc.Bacc()

    XT16 = nc.dram_tensor("XT16", [2 * DIM, BC], F16, kind="ExternalInput")
    W0Tx = nc.dram_tensor("W0Tx", [2 * DIM, H], F16, kind="ExternalInput")
    W0Tv = nc.dram_tensor("W0Tv", [2 * DIM, H], F16, kind="ExternalInput")
    W1T = nc.dram_tensor("W1T", [H, H], F16, kind="ExternalInput")
    W1 = nc.dram_tensor("W1", [H, H], F16, kind="ExternalInput")
    W016 = nc.dram_tensor("W016", [H, 16], F16, kind="ExternalInput")
    P216 = nc.dram_tensor("P216", [H, 16], F16, kind="ExternalInput")
    MMAP = nc.dram_tensor("MMAP", [32, CW], F16, kind="ExternalInput")
    b0c = nc.dram_tensor("b0c", [H, 1], F32, kind="ExternalInput")
    b1c = nc.dram_tensor("b1c", [H, 1], F32, kind="ExternalInput")
    w1c0 = nc.dram_tensor("w1c0", [H, 1], F32, kind="ExternalInput")
    w2n = nc.dram_tensor("w2n", [H, 1], F32, kind="ExternalInput")
    out = nc.dram_tensor("out", [BC, DIM], F32, kind="ExternalOutput")

    from contextlib import ExitStack

    with tile.TileContext(nc) as tc, ExitStack() as stk:
        consts = stk.enter_context(tc.tile_pool(name="consts", bufs=1))
        work = stk.enter_context(tc.tile_pool(name="work", bufs=2))
        tbp = stk.enter_context(tc.tile_pool(name="tbp", bufs=2))
        ps_zz = stk.enter_context(tc.tile_pool(name="ps_zz", bufs=2, space="PSUM"))
        ps_tt = stk.enter_context(tc.tile_pool(name="ps_tt", bufs=2, space="PSUM"))
        ps_aa = stk.enter_context(tc.tile_pool(name="ps_aa", bufs=1, space="PSUM"))
        ps_fm = stk.enter_context(tc.tile_pool(name="ps_fm", bufs=2, space="PSUM"))
        ps_bm = stk.enter_context(tc.tile_pool(name="ps_bm", bufs=1, space="PSUM"))

        # ---------------- constants ----------------
        XT_sb = consts.tile([2 * DIM, BC], F16)
        nc.sync.dma_start(out=XT_sb, in_=XT16[:, :])
        # E_all rows: 0:8 = gn (per tile), 8 = hv, 16:32 = X^T (f16)
        E_all = consts.tile([32, BC], F16)
        nc.sync.dma_start(out=E_all[16:32, :], in_=XT16[:, :])

        W0Tx_sb = consts.tile([2 * DIM, H], F16)
        nc.sync.dma_start(out=W0Tx_sb, in_=W0Tx[:, :])
        W0Tv_sb = consts.tile([2 * DIM, H], F16)
        nc.sync.dma_start(out=W0Tv_sb, in_=W0Tv[:, :])
        W1T_sb = consts.tile([H, H], F16)
        nc.sync.dma_start(out=W1T_sb, in_=W1T[:, :])
        W1_sb = consts.tile([H, H], F16)
        nc.sync.dma_start(out=W1_sb, in_=W1[:, :])
        W016_sb = consts.tile([H, 16], F16)
        nc.sync.dma_start(out=W016_sb, in_=W016[:, :])
        P216_sb = consts.tile([H, 16], F16)
        nc.sync.dma_start(out=P216_sb, in_=P216[:, :])
        M_sb = consts.tile([32, CW], F16)
        nc.sync.dma_start(out=M_sb, in_=MMAP[:, :])
        b0_sb = consts.tile([H, 1], F32)
        nc.sync.dma_start(out=b0_sb, in_=b0c[:, :])
        b1_sb = consts.tile([H, 1], F32)
        nc.sync.dma_start(out=b1_sb, in_=b1c[:, :])
        c0_sb = consts.tile([H, 1], F32)
        nc.sync.dma_start(out=c0_sb, in_=w1c0[:, :])
        w2n_sb = consts.tile([H, 1], F32)
        nc.sync.dma_start(out=w2n_sb, in_=w2n[:, :])

        out_sb = consts.tile([128, (BC // 128) * DIM], F32)

        # ---------------- pipelined main loop ----------------
        state = {}

        def front(t):
            sl = slice(TW * t, TW * (t + 1))

            z0 = ps_zz.tile([H, TW], F32, tag="zz")
            nc.tensor.matmul(z0, W0Tx_sb, XT_sb[:, sl], start=True, stop=True)
            t0 = ps_tt.tile([H, TW], F32, tag="tt")
            nc.tensor.matmul(t0, W0Tv_sb, XT_sb[:, sl], start=True, stop=True)

            h0 = work.tile([H, TW], F16, tag="h0")
            nc.scalar.activation(h0, z0, ACT.Tanh, bias=b0_sb, scale=1.0)

            h0p = work.tile([H, TW], F16, tag="h0p")
            nc.vector._custom_dve(OP_SQM1_MUL, out=h0p, in0=h0, in1=t0[:, :])
            u = work.tile([H, TW], F16, tag="u")
            nc.vector._custom_dve(OP_UPRIME, out=u, in0=h0, in1=t0[:, :])

            z1 = ps_zz.tile([H, TW], F32, tag="zz")
            nc.tensor.matmul(z1, W1T_sb, h0, start=True, stop=True)
            t1 = ps_tt.tile([H, TW], F32, tag="tt")
            nc.tensor.matmul(t1, W1T_sb, h0p, start=True, stop=True)

            h1 = work.tile([H, TW], F16, tag="h1")
            nc.scalar.activation(h1, z1, ACT.Tanh, bias=b1_sb, scale=1.0)

            # A0raw = W1w^T h1sq (w2 and the -1 of d1 folded via c0):
            # A0n = A0raw - c0 = -W1^T a1; subtraction folded into consumers.
            h1sq = work.tile([H, TW], F16, tag="h1sq")
            nc.gpsimd.tensor_mul(h1sq, h1, h1)

            # e1n = -e1  (w2 negated via s0)
            e1 = work.tile([H, TW], F16, tag="e1")
            nc.vector._custom_dve(
                OP_E1F, out=e1, in0=h1, in1=t1[:, :], s0=w2n_sb[:, 0:1]
            )

            A0 = ps_aa.tile([H, TW], F32, tag="aa")
            nc.tensor.matmul(A0, W1_sb, h1sq, start=True, stop=True)

            # a0 = (h0^2-1)*(A0raw-c0) = +(1-h0^2) W1^T a1 (true sign)
            a0 = work.tile([H, TW], F16, tag="a0")
            nc.vector._custom_dve(
                OP_SQM1_MULS, out=a0, in0=h0, in1=A0[:, :], s0=c0_sb[:, 0:1]
            )
            e2 = work.tile([H, TW], F16, tag="e2")
            nc.vector._custom_dve(
                OP_MULS, out=e2, in0=A0[:, :], in1=u, s0=c0_sb[:, 0:1]
            )
            # ecomb = e1n - e2 = -e1 - e2 on GpSimd (plain TT only)
            ecomb = work.tile([H, TW], F16, tag="ec")
            nc.gpsimd.tensor_sub(ecomb, e1, e2)

            # feature-major mini-block: hvv rows 8:16 (start), gn rows 0:8
            # accumulated on top (W016 cols 8:16 are zero, P216 cols 0:8 zero)
            fm = ps_fm.tile([16, TW], F32, tag="fm")
            nc.tensor.matmul(fm, P216_sb, ecomb, start=True, stop=False)
            nc.tensor.matmul(fm, W016_sb, a0, start=False, stop=True)
            state[t] = fm

        def tailstage(t):
            sl = slice(TW * t, TW * (t + 1))
            fm = state.pop(t)
            # stage gn/hv rows next to X^T rows for the fused transpose
            nc.scalar.copy(E_all[0:16, sl], fm)

            # fused transpose + force map: bm[:, 17c+q] per 128-chunk
            bm = ps_bm.tile([128, NCH * CW], F32, tag="bm")
            for c in range(NCH):
                nc.tensor.matmul(
                    bm[:, CW * c : CW * (c + 1)],
                    E_all[:, TW * t + 128 * c : TW * t + 128 * (c + 1)],
                    M_sb,
                    start=True,
                    stop=True,
                )

            g = t // NBT
            dt = t % NBT
            if dt == 0:
                state["tb"] = tbp.tile(
                    [128, NBT * NCH * CW], F32, tag="tb", name="tb4"
                )
            tb4 = state["tb"]
            nc.vector.tensor_copy(
                tb4[:, NCH * CW * dt : NCH * CW * (dt + 1)], bm
            )

            if dt == NBT - 1:
                # ---- batched batch-major tail over NBT tiles ----
                B = NBT * NCH  # 16 chunks
                def col3(off, w):
                    return bass.AP(
                        tensor=tb4.tensor,
                        offset=tb4.offset + off,
                        ap=[list(tb4.ap[0]), [CW, B], [1, w]],
                    )

                f3 = col3(0, DIM)
                gn3 = col3(DIM, DIM)
                hv2 = bass.AP(
                    tensor=tb4.tensor,
                    offset=tb4.offset + 2 * DIM,
                    ap=[list(tb4.ap[0]), [CW, B]],
                )

                gb = tbp.tile([128, 2 * B * DIM], F32, tag="gb")
                gb4 = gb.rearrange("p (q j f) -> p q j f", q=2, f=DIM)
                nc.vector._custom_dve(
                    OP_SQP, out=gb4[:, 0], in0=gn3, imm2=1.0 / DIM
                )
                nc.gpsimd.tensor_mul(gb4[:, 1], gn3, f3)
                red = tbp.tile([128, 2 * B], F32, tag="red")
                red3 = red.rearrange("p (q j) -> p q j", q=2)
                nc.vector.tensor_reduce(red3, gb4, axis=AX.X, op=OP.add)
                num = tbp.tile([128, B], F32, tag="num")
                nc.gpsimd.tensor_sub(num, hv2, red3[:, 1])
                rec = tbp.tile([128, B], F32, tag="rec")
                nc.vector.reciprocal(rec, red3[:, 0])
                s = tbp.tile([128, B], F32, tag="s")
                nc.gpsimd.tensor_mul(s, num, rec)
                sbc = bass.AP(
                    tensor=s.tensor,
                    offset=s.offset,
                    ap=[list(s.ap[0]), [1, B], [0, DIM]],
                )
                su = tbp.tile([128, B * DIM], F32, tag="su")
                su3 = su.rearrange("p (j f) -> p j f", f=DIM)
                nc.gpsimd.tensor_mul(su3, gn3, sbc)
                ob = out_sb[:, B * DIM * g : B * DIM * (g + 1)]
                nc.gpsimd.tensor_add(
                    ob.rearrange("p (j f) -> p j f", f=DIM), f3, su3
                )

        for t in range(NT):
            front(t)
            if t >= 1:
                tailstage(t - 1)
        tailstage(NT - 1)

        nc.sync.dma_start(
            out=out.rearrange("(j p) f -> p j f", p=128),
            in_=out_sb.rearrange("p (j f) -> p j f", f=DIM),
        )

    if not nc.is_finalized():
        nc.finalize()

    return nc


_NC_CACHE = None


def _install_ntff_shim():
    """Register the axon NTFF profile hook (missing antenv.axon_hooks shim)."""
    import sys
    import types

    if "antenv.axon_hooks" in sys.modules:
        return
    try:
        sys.path.insert(0, "/root/.axon_site")
        from trn_agent_boot.trn_boot import _ntff_profile_via_ctypes

        hook = _ntff_profile_via_ctypes("/opt/axon/libaxon_pjrt.so")
        mod = types.ModuleType("antenv.axon_hooks")
        mod.get_axon_ntff_profile_hook = lambda: hook
        sys.modules["antenv.axon_hooks"] = mod
    except Exception:
        pass


def kernel(**inputs):
    global LAST_RESULTS, _NC_CACHE
    trace = bool(int(os.environ.get("KERNEL_TRACE", "0")))
    if trace:
        _install_ntff_shim()
    if _NC_CACHE is None:
        _NC_CACHE = build_nc()
    nc = _NC_CACHE

    X = np.ascontiguousarray(inputs["X"], dtype=np.float32)
    K = np.asarray(inputs["K"], np.float32)
    D = np.asarray(inputs["D"], np.float32)
    W0 = np.asarray(inputs["W0"], np.float32)
    W1 = np.asarray(inputs["W1"], np.float32)
    W2 = np.asarray(inputs["W2"], np.float32)

    w0tx = np.zeros((2 * DIM, H), np.float32)
    w0tx[0:DIM] = W0.T
    w0tv = np.zeros((2 * DIM, H), np.float32)
    w0tv[DIM:] = W0.T
    # M: col q<8: force map (-K^T on x rows 16:24, -D^T on v rows 24:32);
    # cols 8:16 pass gn rows 0:8; col 16 passes hv row 8.
    M = np.zeros((32, CW), np.float32)
    M[16:24, 0:DIM] = -K.T
    M[24:32, 0:DIM] = -D.T
    for i in range(DIM):
        M[i, DIM + i] = 1.0
    M[DIM, 2 * DIM] = -1.0  # hv row negated: num = -hv - g.f

    w016 = np.zeros((H, 16), np.float32)
    w016[:, 0:DIM] = W0
    p216 = np.zeros((H, 16), np.float32)
    p216[:, DIM:] = 2.0

    w1w16 = (W1 * W2.reshape(H, 1)).astype(np.float16)
    c0 = w1w16.astype(np.float32).sum(axis=0).reshape(H, 1).copy()

    shared = {
        "W0Tx": w0tx.astype(np.float16),
        "W0Tv": w0tv.astype(np.float16),
        "W1T": np.ascontiguousarray(W1.T).astype(np.float16),
        "W1": w1w16,
        "W016": w016.astype(np.float16),
        "P216": p216.astype(np.float16),
        "MMAP": M.astype(np.float16),
        "b0c": np.asarray(inputs["b0"], np.float32).reshape(H, 1).copy(),
        "b1c": np.asarray(inputs["b1"], np.float32).reshape(H, 1).copy(),
        "w1c0": c0,
        "w2n": (-W2).reshape(H, 1).copy(),
    }
    in_maps = []
    for i in range(NCORES):
        xt = np.ascontiguousarray(X[i * BC : (i + 1) * BC].T)
        m = {"XT16": xt.astype(np.float16)}
        m.update(shared)
        in_maps.append(m)

    res = run_bass_kernel_spmd(
        nc, in_maps, core_ids=list(range(NCORES)), trace=trace
    )
    LAST_RESULTS = res
    out_full = np.concatenate([res.results[i]["out"] for i in range(NCORES)], axis=0)
    return out_full.astype(np.float32)


# revision 33
# speedup vs baseline: 1.3449x; 1.1178x over previous
"""Trainium2 Bass kernel for the nn_Dynamics problem.

Math (per batch element, d=8, H=128):
  x = X[:, :8], v = X[:, 8:]
  z0 = W0 x + b0; h0 = tanh(z0); z1 = W1 h0 + b1; h1 = tanh(z1)
  a1 = (1-h1^2)*w2;  A0 = W1^T a1;  a0 = (1-h0^2)*A0;  g = W0^T a0
  t0 = W0 v; t1 = W1((1-h0^2) t0)
  hvv = -2 sum_k [a1 h1 t1^2 + A0 h0 (1-h0^2) t0^2]
  force = -(K x + D v)
  out = force - g*(g.force + hvv)/(1 + |g|^2)      (Sherman-Morrison)

Device mapping:
  - Host pre-transposes X (f16), with a batch permutation (col 128J+p holds
    X row 64p+J) so the final out DMA is 2KB-contiguous per partition.
  - w2 and the -1 of d1=(1-h1^2) are folded into the A0 stationary:
    A0raw = (W1*w2)^T h1sq, A0n = A0raw - c0 = -W1^T a1;  c0 column is
    subtracted inside the custom consumer ops.
  - e1n = -e1 (w2 negated), ecomb = e1n - e2 (GpSimd), hv = sum_k(ecomb)
    via GpSimd partition_all_reduce (= hvv/2, sign folded into M).
  - g via one m=8 PE matmul; per-tile g rows + hv row staged next to X^T
    rows in E_all (f16); per 4-tile group, 16 tiny transpose-matmuls with
    the augmented moving map M compute [force, g, hv] batch-major in one
    psum bank; one scalar copy stages it for the batch-major tail.
Sharding: pure data parallel over 8 NeuronCores (8192 rows each).
"""

import os

import numpy as np

import concourse.bacc as bacc
import concourse.bass as bass
import concourse.bass_isa as bass_isa
import concourse.dve_ops as dve_ops
import concourse.tile as tile
from concourse import mybir
from concourse.bass_utils import run_bass_kernel_spmd
from concourse.dve_ops import DveOp
from concourse.dve_ops import has_src1
from concourse.dve_spec import C0, C2, One, Spec, Src0, Src1, lower, sq
from concourse.dve_uop import DveOpSpec

F32 = mybir.dt.float32
F16 = mybir.dt.float16
AX = mybir.AxisListType
OP = mybir.AluOpType
ACT = mybir.ActivationFunctionType

DIM = 8
H = 128
BATCH = 65536
NCORES = 8
BC = BATCH // NCORES          # 8192 rows per core
TW = 512                      # batch tile width
NT = BC // TW                 # 16 tiles per core
NCH = TW // 128               # 4 chunks of 128 per tile
NBT = 4                       # tiles per tail group
NG = NT // NBT                # tail groups
CW = 2 * DIM + 1              # 17 packed batch-major cols per chunk
GW = NBT * NCH * CW           # 272 bm cols per group

LAST_RESULTS = None

# ---------------- custom fused DVE ops ----------------


def _register_op(name, body, reference):
    if name in dve_ops._SUB_OPCODE_FOR_NAME:
        for op in dve_ops.OPS:
            if op.name == name:
                return op
    spec = Spec(body=body, reference=reference)
    shas = {}
    for ver in ("v3", "v4"):
        shas[ver] = DveOpSpec(
            name=name,
            opcode=dve_ops._CUSTOM_DVE_ROW_BASE + len(dve_ops.OPS),
            uops=lower(spec, ver=ver),
            rd1_en=has_src1(spec),
        ).sha(ver)
    op = DveOp(name, spec, subdim=False, uops_sha=shas)
    dve_ops.OPS.append(op)
    dve_ops.CUSTOM_DVE_SPECS[name] = spec
    dve_ops._SUB_OPCODE_FOR_NAME[name] = (
        dve_ops._CUSTOM_DVE_ROW_BASE + len(dve_ops.OPS) - 1
    )
    return op


# h0p' = (h0^2 - 1) * t0
OP_SQM1_MUL = _register_op(
    "ANT_SQM1_MUL",
    (sq(Src0) - One) * Src1,
    lambda in0, in1: (in0 * in0 - 1.0) * in1,
)
# u' = h0 * (h0^2 - 1) * t0^2
OP_UPRIME = _register_op(
    "ANT_UPRIME",
    Src0 * (sq(Src0) - One) * sq(Src1),
    lambda in0, in1: in0 * (in0 * in0 - 1.0) * in1 * in1,
)
# e1 = (1 - h1^2) * w2 * h1 * t1^2
OP_E1F = _register_op(
    "ANT_E1F",
    (One - sq(Src0)) * C0 * Src0 * sq(Src1),
    lambda in0, in1, s0: (1.0 - in0 * in0) * s0 * in0 * in1 * in1,
)
# gsq with 1/8 folded in: sum over 8 features gives 1+|g|^2 directly
OP_SQP = _register_op(
    "ANT_SQP",
    sq(Src0) + C2,
    lambda in0, imm2: in0 * in0 + imm2,
)
# a0 = (h0^2-1) * (A0raw - c0)
OP_SQM1_MULS = _register_op(
    "ANT_SQM1_MULS",
    (sq(Src0) - One) * (Src1 - C0),
    lambda in0, in1, s0: (in0 * in0 - 1.0) * (in1 - s0),
)
# e2 = (A0raw - c0) * u
OP_MULS = _register_op(
    "ANT_MULS",
    (Src0 - C0) * Src1,
    lambda in0, in1, s0: (in0 - s0) * in1,
)

# f16 weight blob layout (free-axis offsets)
B_W0TX = 0          # [16, 128]
B_W0TV = 128        # [16, 128]
B_M = 256           # [32, 17]
B_W1T = 288         # [128, 128]
B_W1W = 416         # [128, 128]
B_W016 = 544        # [128, 16] (W0 | zeros)
B_P216 = 560        # [128, 16] (zeros | 2.0)
B_F16 = 576


def build_nc():
    nc = bacc.Bacc()

    XT16 = nc.dram_tensor("XT16", [2 * DIM, BC], F16, kind="ExternalInput")
    WB16 = nc.dram_tensor("WB16", [128, B_F16], F16, kind="ExternalInput")
    WB32 = nc.dram_tensor("WB32", [128, 4], F32, kind="ExternalInput")
    out = nc.dram_tensor("out", [BC, DIM], F32, kind="ExternalOutput")

    from contextlib import ExitStack

    with tile.TileContext(nc) as tc, ExitStack() as stk:
        consts = stk.enter_context(tc.tile_pool(name="consts", bufs=1))
        work = stk.enter_context(tc.tile_pool(name="work", bufs=2))
        tbp = stk.enter_context(tc.tile_pool(name="tbp", bufs=2))
        ps_zz = stk.enter_context(tc.tile_pool(name="ps_zz", bufs=2, space="PSUM"))
        ps_tt = stk.enter_context(tc.tile_pool(name="ps_tt", bufs=2, space="PSUM"))
        ps_aa = stk.enter_context(tc.tile_pool(name="ps_aa", bufs=1, space="PSUM"))
        ps_fm = stk.enter_context(tc.tile_pool(name="ps_fm", bufs=1, space="PSUM"))
        ps_bm = stk.enter_context(tc.tile_pool(name="ps_bm", bufs=2, space="PSUM"))

        # ---------------- constants ----------------
        XT_sb = consts.tile([2 * DIM, BC], F16)
        nc.sync.dma_start(out=XT_sb[:, 0 : 2 * TW], in_=XT16[:, 0 : 2 * TW])
        nc.sync.dma_start(out=XT_sb[:, 2 * TW :], in_=XT16[:, 2 * TW :])
        # E_all rows: 0:8 = g (per tile), 8 = hv, 16:32 = X^T (f16)
        E_all = consts.tile([32, BC], F16)
        nc.sync.dma_start(out=E_all[16:32, :], in_=XT16[:, :])

        wb16 = consts.tile([128, B_F16], F16)
        nc.sync.dma_start(out=wb16, in_=WB16[:, :])
        wb32 = consts.tile([128, 4], F32)
        nc.sync.dma_start(out=wb32, in_=WB32[:, :])

        W0Tx_sb = wb16[0:16, B_W0TX : B_W0TX + 128]
        W0Tv_sb = wb16[0:16, B_W0TV : B_W0TV + 128]
        M_sb = wb16[0:32, B_M : B_M + CW]
        W1T_sb = wb16[:, B_W1T : B_W1T + 128]
        W1w_sb = wb16[:, B_W1W : B_W1W + 128]
        W016_sb = wb16[:, B_W016 : B_W016 + 16]
        P216_sb = wb16[:, B_P216 : B_P216 + 16]
        b0_sb = wb32[:, 0:1]
        b1_sb = wb32[:, 1:2]
        c0_sb = wb32[:, 2:3]
        w2n_sb = wb32[:, 3:4]

        out_sb = consts.tile([128, (BC // 128) * DIM], F32)

        # ---------------- pipelined main loop ----------------
        state = {}

        def front(t):
            sl = slice(TW * t, TW * (t + 1))

            z0 = ps_zz.tile([H, TW], F32, tag="zz")
            nc.tensor.matmul(z0, W0Tx_sb, XT_sb[:, sl], start=True, stop=True)
            t0 = ps_tt.tile([H, TW], F32, tag="tt")
            nc.tensor.matmul(t0, W0Tv_sb, XT_sb[:, sl], start=True, stop=True)

            h0 = work.tile([H, TW], F16, tag="h0")
            nc.scalar.activation(h0, z0, ACT.Tanh, bias=b0_sb, scale=1.0)

            h0p = work.tile([H, TW], F16, tag="h0p")
            nc.vector._custom_dve(OP_SQM1_MUL, out=h0p, in0=h0, in1=t0[:, :])
            u = work.tile([H, TW], F16, tag="u")
            nc.vector._custom_dve(OP_UPRIME, out=u, in0=h0, in1=t0[:, :])

            z1 = ps_zz.tile([H, TW], F32, tag="zz")
            nc.tensor.matmul(z1, W1T_sb, h0, start=True, stop=True)
            t1 = ps_tt.tile([H, TW], F32, tag="tt")
            nc.tensor.matmul(t1, W1T_sb, h0p, start=True, stop=True)

            h1 = work.tile([H, TW], F16, tag="h1")
            nc.scalar.activation(h1, z1, ACT.Tanh, bias=b1_sb, scale=1.0)

            # h1sq on the Activation engine (same act table as tanh)
            h1sq = work.tile([H, TW], F16, tag="h1sq")
            nc.scalar.activation(h1sq, h1, ACT.Square)

            # e1n = -e1  (w2 negated via s0)
            e1 = work.tile([H, TW], F16, tag="e1")
            nc.vector._custom_dve(
                OP_E1F, out=e1, in0=h1, in1=t1[:, :], s0=w2n_sb
            )

            A0 = ps_aa.tile([H, TW], F32, tag="aa")
            nc.tensor.matmul(A0, W1w_sb, h1sq, start=True, stop=True)

            # a0 = (h0^2-1)*(A0raw-c0) = +(1-h0^2) W1^T a1 (true sign)
            a0 = work.tile([H, TW], F16, tag="a0")
            nc.vector._custom_dve(
                OP_SQM1_MULS, out=a0, in0=h0, in1=A0[:, :], s0=c0_sb
            )
            e2 = work.tile([H, TW], F16, tag="e2")
            nc.vector._custom_dve(
                OP_MULS, out=e2, in0=A0[:, :], in1=u, s0=c0_sb
            )
            # ecomb = e1n - e2 = -e1 - e2 on GpSimd (plain TT only)
            ecomb = work.tile([H, TW], F16, tag="ec")
            nc.gpsimd.tensor_sub(ecomb, e1, e2)

            # feature-major mini-block: hv rows 8:16 (start), g rows 0:8
            # accumulated on top (W016 cols 8:16 zero, P216 cols 0:8 zero)
            fm = ps_fm.tile([16, TW], F32, tag="fm")
            nc.tensor.matmul(fm, P216_sb, ecomb, start=True, stop=False)
            nc.tensor.matmul(fm, W016_sb, a0, start=False, stop=True)
            state[t] = fm

        def stage2(t):
            sl = slice(TW * t, TW * (t + 1))
            fm = state.pop(t)
            # stage g/hv rows next to X^T rows for the fused transpose
            nc.scalar.copy(E_all[0:16, sl], fm)

        def groupstage(g):
            # fused transpose + force map: one psum bank per group
            bm = ps_bm.tile([128, GW], F32, tag="bm")
            for j in range(NBT * NCH):
                nc.tensor.matmul(
                    bm[:, CW * j : CW * (j + 1)],
                    E_all[:, NBT * TW * g + 128 * j : NBT * TW * g + 128 * (j + 1)],
                    M_sb,
                    start=True,
                    stop=True,
                )
            tb4 = tbp.tile([128, GW], F32, tag="tb", name="tb4")
            nc.scalar.copy(tb4, bm)

            # ---- batched batch-major tail ----
            B = NBT * NCH  # 16 chunks
            def col3(off, w):
                return bass.AP(
                    tensor=tb4.tensor,
                    offset=tb4.offset + off,
                    ap=[list(tb4.ap[0]), [CW, B], [1, w]],
                )

            f3 = col3(0, DIM)
            gn3 = col3(DIM, DIM)
            hv2 = bass.AP(
                tensor=tb4.tensor,
                offset=tb4.offset + 2 * DIM,
                ap=[list(tb4.ap[0]), [CW, B]],
            )

            gb = tbp.tile([128, 2 * B * DIM], F32, tag="gb")
            gb4 = gb.rearrange("p (q j f) -> p q j f", q=2, f=DIM)
            nc.vector._custom_dve(OP_SQP, out=gb4[:, 0], in0=gn3, imm2=1.0 / DIM)
            nc.gpsimd.tensor_mul(gb4[:, 1], gn3, f3)
            red = tbp.tile([128, 2 * B], F32, tag="red")
            red3 = red.rearrange("p (q j) -> p q j", q=2)
            nc.vector.tensor_reduce(red3, gb4, axis=AX.X, op=OP.add)
            num = tbp.tile([128, B], F32, tag="num")
            nc.vector.tensor_sub(num, hv2, red3[:, 1])
            rec = tbp.tile([128, B], F32, tag="rec")
            nc.vector.reciprocal(rec, red3[:, 0])
            s = tbp.tile([128, B], F32, tag="s")
            nc.vector.tensor_mul(s, num, rec)
            sbc = bass.AP(
                tensor=s.tensor,
                offset=s.offset,
                ap=[list(s.ap[0]), [1, B], [0, DIM]],
            )
            su = tbp.tile([128, B * DIM], F32, tag="su")
            su3 = su.rearrange("p (j f) -> p j f", f=DIM)
            nc.gpsimd.tensor_mul(su3, gn3, sbc)
            ob = out_sb[:, B * DIM * g : B * DIM * (g + 1)]
            nc.gpsimd.tensor_add(
                ob.rearrange("p (j f) -> p j f", f=DIM), f3, su3
            )

        for t in range(NT):
            front(t)
            if t >= 1:
                stage2(t - 1)
            if t % NBT == 0 and t >= NBT:
                groupstage(t // NBT - 1)
        stage2(NT - 1)
        groupstage(NG - 1)

        nc.sync.dma_start(
            out=out.rearrange("(p j) f -> p (j f)", p=128), in_=out_sb
        )

    if not nc.is_finalized():
        nc.finalize()

    return nc


_NC_CACHE = None


def _install_ntff_shim():
    """Register the axon NTFF profile hook (missing antenv.axon_hooks shim)."""
    import sys
    import types

    if "antenv.axon_hooks" in sys.modules:
        return
    try:
        sys.path.insert(0, "/root/.axon_site")
        from trn_agent_boot.trn_boot import _ntff_profile_via_ctypes

        hook = _ntff_profile_via_ctypes("/opt/axon/libaxon_pjrt.so")
        mod = types.ModuleType("antenv.axon_hooks")
        mod.get_axon_ntff_profile_hook = lambda: hook
        sys.modules["antenv.axon_hooks"] = mod
    except Exception:
        pass


def kernel(**inputs):
    global LAST_RESULTS, _NC_CACHE
    trace = bool(int(os.environ.get("KERNEL_TRACE", "0")))
    if trace:
        _install_ntff_shim()
    if _NC_CACHE is None:
        _NC_CACHE = build_nc()
    nc = _NC_CACHE

    X = np.ascontiguousarray(inputs["X"], dtype=np.float32)
    K = np.asarray(inputs["K"], np.float32)
    D = np.asarray(inputs["D"], np.float32)
    W0 = np.asarray(inputs["W0"], np.float32)
    W1 = np.asarray(inputs["W1"], np.float32)
    W2 = np.asarray(inputs["W2"], np.float32)

    w1w16 = (W1 * W2.reshape(H, 1)).astype(np.float16)
    c0 = w1w16.astype(np.float32).sum(axis=0).reshape(H, 1)

    wb16 = np.zeros((128, B_F16), np.float32)
    wb16[0:DIM, B_W0TX : B_W0TX + 128] = W0.T
    wb16[DIM : 2 * DIM, B_W0TV : B_W0TV + 128] = W0.T
    # M: col q<8: force map (-K^T on x rows 16:24, -D^T on v rows 24:32);
    # cols 8:16 pass g rows 0:8; col 16 = -2*hv row 8 (hv = hvv/2, negated).
    wb16[16:24, B_M : B_M + DIM] = -K.T
    wb16[24:32, B_M : B_M + DIM] = -D.T
    for i in range(DIM):
        wb16[i, B_M + DIM + i] = 1.0
    wb16[DIM, B_M + 2 * DIM] = -1.0  # hv_row = 2*sum(ecomb) = hvv; negate
    wb16[:, B_W1T : B_W1T + 128] = W1.T
    wb16[:, B_W1W : B_W1W + 128] = w1w16.astype(np.float32)
    wb16[:, B_W016 : B_W016 + DIM] = W0
    wb16[:, B_P216 + DIM : B_P216 + 16] = 2.0

    wb32 = np.zeros((128, 4), np.float32)
    wb32[:, 0] = np.asarray(inputs["b0"], np.float32)
    wb32[:, 1] = np.asarray(inputs["b1"], np.float32)
    wb32[:, 2] = c0[:, 0]
    wb32[:, 3] = -W2.reshape(H)

    shared = {"WB16": wb16.astype(np.float16), "WB32": wb32}

    # batch permutation: device column 128J+p holds X row 64p+J so the
    # output DMA is 2KB-contiguous per partition.
    b = np.arange(BC)
    perm = (BC // 128) * (b % 128) + b // 128

    in_maps = []
    for i in range(NCORES):
        xp = X[i * BC : (i + 1) * BC][perm]
        m = {"XT16": np.ascontiguousarray(xp.T).astype(np.float16)}
        m.update(shared)
        in_maps.append(m)

    res = run_bass_kernel_spmd(
        nc, in_maps, core_ids=list(range(NCORES)), trace=trace
    )
    LAST_RESULTS = res
    out_full = np.concatenate([res.results[i]["out"] for i in range(NCORES)], axis=0)
    return out_full.astype(np.float32)


# revision 37
# speedup vs baseline: 1.5051x; 1.1191x over previous
"""Trainium2 Bass kernel for the nn_Dynamics problem.

Math (per batch element, d=8, H=128):
  x = X[:, :8], v = X[:, 8:]
  z0 = W0 x + b0; h0 = tanh(z0); z1 = W1 h0 + b1; h1 = tanh(z1)
  a1 = (1-h1^2)*w2;  A0 = W1^T a1;  a0 = (1-h0^2)*A0;  g = W0^T a0
  t0 = W0 v; t1 = W1((1-h0^2) t0)
  hvv = -2 sum_k [a1 h1 t1^2 + A0 h0 (1-h0^2) t0^2]
  force = -(K x + D v)
  out = force - g*(g.force + hvv)/(1 + |g|^2)      (Sherman-Morrison)

Device mapping:
  - Host pre-transposes X (f16), with a batch permutation (col 128J+p holds
    X row 64p+J) so the final out DMA is 2KB-contiguous per partition.
  - w2 and the -1 of d1=(1-h1^2) are folded into the A0 stationary:
    A0raw = (W1*w2)^T h1sq, A0n = A0raw - c0 = -W1^T a1;  c0 column is
    subtracted inside the custom consumer ops.
  - e1n = -e1 (w2 negated), ecomb = e1n - e2 (GpSimd), hv = sum_k(ecomb)
    via GpSimd partition_all_reduce (= hvv/2, sign folded into M).
  - g via one m=8 PE matmul; per-tile g rows + hv row staged next to X^T
    rows in E_all (f16); per 4-tile group, 16 tiny transpose-matmuls with
    the augmented moving map M compute [force, g, hv] batch-major in one
    psum bank; one scalar copy stages it for the batch-major tail.
Sharding: pure data parallel over 8 NeuronCores (8192 rows each).
"""

import os

import numpy as np

import concourse.bacc as bacc
import concourse.bass as bass
import concourse.bass_isa as bass_isa
import concourse.dve_ops as dve_ops
import concourse.tile as tile
from concourse import mybir
from concourse.bass_utils import run_bass_kernel_spmd
from concourse.dve_ops import DveOp
from concourse.dve_ops import has_src1
from concourse.dve_spec import C0, C2, One, Spec, Src0, Src1, lower, sq
from concourse.dve_uop import DveOpSpec

F32 = mybir.dt.float32
F16 = mybir.dt.float16
AX = mybir.AxisListType
OP = mybir.AluOpType
ACT = mybir.ActivationFunctionType

DIM = 8
H = 128
BATCH = 65536
NCORES = 8
BC = BATCH // NCORES          # 8192 rows per core
TW = 512                      # batch tile width
NT = BC // TW                 # 16 tiles per core
NCH = TW // 128               # 4 chunks of 128 per tile
NBT = 4                       # tiles per tail group
NG = NT // NBT                # tail groups
CW = 2 * DIM + 1              # 17 packed batch-major cols per chunk
GW = NBT * NCH * CW           # 272 bm cols per group

LAST_RESULTS = None

# ---------------- custom fused DVE ops ----------------


def _register_op(name, body, reference):
    if name in dve_ops._SUB_OPCODE_FOR_NAME:
        for op in dve_ops.OPS:
            if op.name == name:
                return op
    spec = Spec(body=body, reference=reference)
    shas = {}
    for ver in ("v3", "v4"):
        shas[ver] = DveOpSpec(
            name=name,
            opcode=dve_ops._CUSTOM_DVE_ROW_BASE + len(dve_ops.OPS),
            uops=lower(spec, ver=ver),
            rd1_en=has_src1(spec),
        ).sha(ver)
    op = DveOp(name, spec, subdim=False, uops_sha=shas)
    dve_ops.OPS.append(op)
    dve_ops.CUSTOM_DVE_SPECS[name] = spec
    dve_ops._SUB_OPCODE_FOR_NAME[name] = (
        dve_ops._CUSTOM_DVE_ROW_BASE + len(dve_ops.OPS) - 1
    )
    return op


# h0p' = (h0^2 - 1) * t0
OP_SQM1_MUL = _register_op(
    "ANT_SQM1_MUL",
    (sq(Src0) - One) * Src1,
    lambda in0, in1: (in0 * in0 - 1.0) * in1,
)
# u' = h0 * (h0^2 - 1) * t0^2
OP_UPRIME = _register_op(
    "ANT_UPRIME",
    Src0 * (sq(Src0) - One) * sq(Src1),
    lambda in0, in1: in0 * (in0 * in0 - 1.0) * in1 * in1,
)
# e1 = (1 - h1^2) * w2 * h1 * t1^2
OP_E1F = _register_op(
    "ANT_E1F",
    (One - sq(Src0)) * C0 * Src0 * sq(Src1),
    lambda in0, in1, s0: (1.0 - in0 * in0) * s0 * in0 * in1 * in1,
)
# gsq with 1/8 folded in: sum over 8 features gives 1+|g|^2 directly
OP_SQP = _register_op(
    "ANT_SQP",
    sq(Src0) + C2,
    lambda in0, imm2: in0 * in0 + imm2,
)
# a0 = (h0^2-1) * (A0raw - c0)
OP_SQM1_MULS = _register_op(
    "ANT_SQM1_MULS",
    (sq(Src0) - One) * (Src1 - C0),
    lambda in0, in1, s0: (in0 * in0 - 1.0) * (in1 - s0),
)
# e2 = (A0raw - c0) * u
OP_MULS = _register_op(
    "ANT_MULS",
    (Src0 - C0) * Src1,
    lambda in0, in1, s0: (in0 - s0) * in1,
)

# f16 weight blob layout (free-axis offsets)
B_W0TX = 0          # [16, 128]
B_W0TV = 128        # [16, 128]
B_M = 256           # [32, 17]
B_W1T = 288         # [128, 128]
B_W1W = 416         # [128, 128]
B_W016 = 544        # [128, 16] (W0 | zeros)
B_P216 = 560        # [128, 16] (zeros | 2.0)
B_F16 = 576


def build_nc():
    nc = bacc.Bacc()

    XT16 = nc.dram_tensor("XT16", [2 * DIM, BC], F16, kind="ExternalInput")
    WB16 = nc.dram_tensor("WB16", [128, B_F16], F16, kind="ExternalInput")
    WB32 = nc.dram_tensor("WB32", [128, 4], F32, kind="ExternalInput")
    out = nc.dram_tensor("out", [BC, DIM], F32, kind="ExternalOutput")

    from contextlib import ExitStack

    with tile.TileContext(nc) as tc, ExitStack() as stk:
        consts = stk.enter_context(tc.tile_pool(name="consts", bufs=1))
        work = stk.enter_context(tc.tile_pool(name="work", bufs=2))
        tbp = stk.enter_context(tc.tile_pool(name="tbp", bufs=2))
        ps_zz = stk.enter_context(tc.tile_pool(name="ps_zz", bufs=2, space="PSUM"))
        ps_tt = stk.enter_context(tc.tile_pool(name="ps_tt", bufs=2, space="PSUM"))
        ps_aa = stk.enter_context(tc.tile_pool(name="ps_aa", bufs=2, space="PSUM"))
        ps_fm = stk.enter_context(tc.tile_pool(name="ps_fm", bufs=1, space="PSUM"))
        ps_bm = stk.enter_context(tc.tile_pool(name="ps_bm", bufs=1, space="PSUM"))

        # ---------------- constants (weights first: first z0 needs them) ----
        wb16 = consts.tile([128, B_F16], F16)
        nc.sync.dma_start(out=wb16, in_=WB16[:, :])
        wb32 = consts.tile([128, 4], F32)
        nc.sync.dma_start(out=wb32, in_=WB32[:, :])

        XT_sb = consts.tile([2 * DIM, BC], F16)
        nc.sync.dma_start(out=XT_sb[:, 0 : 2 * TW], in_=XT16[:, 0 : 2 * TW])
        nc.sync.dma_start(out=XT_sb[:, 2 * TW :], in_=XT16[:, 2 * TW :])
        # E_all rows: 0:8 = g (per tile), 8 = hv, 16:32 = X^T (f16)
        E_all = consts.tile([32, BC], F16)
        nc.sync.dma_start(out=E_all[16:32, :], in_=XT16[:, :])

        W0Tx_sb = wb16[0:16, B_W0TX : B_W0TX + 128]
        W0Tv_sb = wb16[0:16, B_W0TV : B_W0TV + 128]
        M_sb = wb16[0:32, B_M : B_M + CW]
        W1T_sb = wb16[:, B_W1T : B_W1T + 128]
        W1w_sb = wb16[:, B_W1W : B_W1W + 128]
        W016_sb = wb16[:, B_W016 : B_W016 + 16]
        P216_sb = wb16[:, B_P216 : B_P216 + 16]
        b0_sb = wb32[:, 0:1]
        b1_sb = wb32[:, 1:2]
        c0_sb = wb32[:, 2:3]
        w2n_sb = wb32[:, 3:4]

        out_sb = consts.tile([128, (BC // 128) * DIM], F32)

        # ---------------- pipelined main loop ----------------
        state = {}

        def front(t):
            sl = slice(TW * t, TW * (t + 1))

            z0 = ps_zz.tile([H, TW], F32, tag="zz")
            nc.tensor.matmul(z0, W0Tx_sb, XT_sb[:, sl], start=True, stop=True)
            t0 = ps_tt.tile([H, TW], F32, tag="tt")
            nc.tensor.matmul(t0, W0Tv_sb, XT_sb[:, sl], start=True, stop=True)

            h0 = work.tile([H, TW], F16, tag="h0")
            nc.scalar.activation(h0, z0, ACT.Tanh, bias=b0_sb, scale=1.0)

            h0p = work.tile([H, TW], F16, tag="h0p")
            nc.vector._custom_dve(OP_SQM1_MUL, out=h0p, in0=h0, in1=t0[:, :])
            u = work.tile([H, TW], F16, tag="u")
            nc.vector._custom_dve(OP_UPRIME, out=u, in0=h0, in1=t0[:, :])

            z1 = ps_zz.tile([H, TW], F32, tag="zz")
            nc.tensor.matmul(z1, W1T_sb, h0, start=True, stop=True)
            t1 = ps_tt.tile([H, TW], F32, tag="tt")
            nc.tensor.matmul(t1, W1T_sb, h0p, start=True, stop=True)

            h1 = work.tile([H, TW], F16, tag="h1")
            nc.scalar.activation(h1, z1, ACT.Tanh, bias=b1_sb, scale=1.0)

            # h1sq on the Activation engine (same act table as tanh)
            h1sq = work.tile([H, TW], F16, tag="h1sq")
            nc.scalar.activation(h1sq, h1, ACT.Square)

            # e1n = -e1  (w2 negated via s0)
            e1 = work.tile([H, TW], F16, tag="e1")
            nc.vector._custom_dve(
                OP_E1F, out=e1, in0=h1, in1=t1[:, :], s0=w2n_sb
            )

            A0 = ps_aa.tile([H, TW], F32, tag="aa")
            nc.tensor.matmul(A0, W1w_sb, h1sq, start=True, stop=True)

            # a0 = (h0^2-1)*(A0raw-c0) = +(1-h0^2) W1^T a1 (true sign)
            a0 = work.tile([H, TW], F16, tag="a0")
            nc.vector._custom_dve(
                OP_SQM1_MULS, out=a0, in0=h0, in1=A0[:, :], s0=c0_sb
            )
            e2 = work.tile([H, TW], F16, tag="e2")
            nc.vector._custom_dve(
                OP_MULS, out=e2, in0=A0[:, :], in1=u, s0=c0_sb
            )
            # ecomb = e1n - e2 = -e1 - e2 on GpSimd (plain TT only)
            ecomb = work.tile([H, TW], F16, tag="ec")
            nc.gpsimd.tensor_sub(ecomb, e1, e2)

            # feature-major mini-block: g rows 0:8 first (a0 ready early),
            # hv rows 8:16 accumulated on top (disjoint stationary columns)
            fm = ps_fm.tile([16, TW], F32, tag="fm")
            nc.tensor.matmul(fm, W016_sb, a0, start=True, stop=False)
            nc.tensor.matmul(fm, P216_sb, ecomb, start=False, stop=True)
            state[t] = fm

        def stage2(t):
            sl = slice(TW * t, TW * (t + 1))
            fm = state.pop(t)
            # stage g/hv rows next to X^T rows for the fused transpose
            nc.scalar.copy(E_all[0:16, sl], fm)

        def groupstage(g):
            # fused transpose + force map: one psum bank per group
            bm = ps_bm.tile([128, GW], F32, tag="bm")
            for j in range(NBT * NCH):
                nc.tensor.matmul(
                    bm[:, CW * j : CW * (j + 1)],
                    E_all[:, NBT * TW * g + 128 * j : NBT * TW * g + 128 * (j + 1)],
                    M_sb,
                    start=True,
                    stop=True,
                )
            tb4 = tbp.tile([128, GW], F32, tag="tb", name="tb4")
            nc.scalar.copy(tb4, bm)

            # ---- batched batch-major tail ----
            B = NBT * NCH  # 16 chunks
            def col3(off, w):
                return bass.AP(
                    tensor=tb4.tensor,
                    offset=tb4.offset + off,
                    ap=[list(tb4.ap[0]), [CW, B], [1, w]],
                )

            f3 = col3(0, DIM)
            gn3 = col3(DIM, DIM)
            hv2 = bass.AP(
                tensor=tb4.tensor,
                offset=tb4.offset + 2 * DIM,
                ap=[list(tb4.ap[0]), [CW, B]],
            )

            gb = tbp.tile([128, 2 * B * DIM], F32, tag="gb")
            gb4 = gb.rearrange("p (q j f) -> p q j f", q=2, f=DIM)
            nc.vector._custom_dve(OP_SQP, out=gb4[:, 0], in0=gn3, imm2=1.0 / DIM)
            nc.gpsimd.tensor_mul(gb4[:, 1], gn3, f3)
            red = tbp.tile([128, 2 * B], F32, tag="red")
            red3 = red.rearrange("p (q j) -> p q j", q=2)
            nc.vector.tensor_reduce(red3, gb4, axis=AX.X, op=OP.add)
            num = tbp.tile([128, B], F32, tag="num")
            nc.vector.tensor_sub(num, hv2, red3[:, 1])
            rec = tbp.tile([128, B], F32, tag="rec")
            nc.vector.reciprocal(rec, red3[:, 0])
            s = tbp.tile([128, B], F32, tag="s")
            nc.vector.tensor_mul(s, num, rec)
            sbc = bass.AP(
                tensor=s.tensor,
                offset=s.offset,
                ap=[list(s.ap[0]), [1, B], [0, DIM]],
            )
            su = tbp.tile([128, B * DIM], F32, tag="su")
            su3 = su.rearrange("p (j f) -> p j f", f=DIM)
            nc.gpsimd.tensor_mul(su3, gn3, sbc)
            ob = out_sb[:, B * DIM * g : B * DIM * (g + 1)]
            nc.gpsimd.tensor_add(
                ob.rearrange("p (j f) -> p j f", f=DIM), f3, su3
            )
            # stream this group's output slice out immediately
            nc.sync.dma_start(
                out=out.rearrange("(p j) f -> p (j f)", p=128)[
                    :, B * DIM * g : B * DIM * (g + 1)
                ],
                in_=ob,
            )

        for t in range(NT):
            front(t)
            if t >= 1:
                stage2(t - 1)
            if t % NBT == 0 and t >= NBT:
                groupstage(t // NBT - 1)
        stage2(NT - 1)
        groupstage(NG - 1)

    if not nc.is_finalized():
        nc.finalize()

    return nc


_NC_CACHE = None


def _install_ntff_shim():
    """Register the axon NTFF profile hook (missing antenv.axon_hooks shim)."""
    import sys
    import types

    if "antenv.axon_hooks" in sys.modules:
        return
    try:
        sys.path.insert(0, "/root/.axon_site")
        from trn_agent_boot.trn_boot import _ntff_profile_via_ctypes

        hook = _ntff_profile_via_ctypes("/opt/axon/libaxon_pjrt.so")
        mod = types.ModuleType("antenv.axon_hooks")
        mod.get_axon_ntff_profile_hook = lambda: hook
        sys.modules["antenv.axon_hooks"] = mod
    except Exception:
        pass


def kernel(**inputs):
    global LAST_RESULTS, _NC_CACHE
    trace = bool(int(os.environ.get("KERNEL_TRACE", "0")))
    if trace:
        _install_ntff_shim()
    if _NC_CACHE is None:
        _NC_CACHE = build_nc()
    nc = _NC_CACHE

    X = np.ascontiguousarray(inputs["X"], dtype=np.float32)
    K = np.asarray(inputs["K"], np.float32)
    D = np.asarray(inputs["D"], np.float32)
    W0 = np.asarray(inputs["W0"], np.float32)
    W1 = np.asarray(inputs["W1"], np.float32)
    W2 = np.asarray(inputs["W2"], np.float32)

    w1w16 = (W1 * W2.reshape(H, 1)).astype(np.float16)
    c0 = w1w16.astype(np.float32).sum(axis=0).reshape(H, 1)

    wb16 = np.zeros((128, B_F16), np.float32)
    wb16[0:DIM, B_W0TX : B_W0TX + 128] = W0.T
    wb16[DIM : 2 * DIM, B_W0TV : B_W0TV + 128] = W0.T
    # M: col q<8: force map (-K^T on x rows 16:24, -D^T on v rows 24:32);
    # cols 8:16 pass g rows 0:8; col 16 = -2*hv row 8 (hv = hvv/2, negated).
    wb16[16:24, B_M : B_M + DIM] = -K.T
    wb16[24:32, B_M : B_M + DIM] = -D.T
    for i in range(DIM):
        wb16[i, B_M + DIM + i] = 1.0
    wb16[DIM, B_M + 2 * DIM] = -1.0  # hv_row = 2*sum(ecomb) = hvv; negate
    wb16[:, B_W1T : B_W1T + 128] = W1.T
    wb16[:, B_W1W : B_W1W + 128] = w1w16.astype(np.float32)
    wb16[:, B_W016 : B_W016 + DIM] = W0
    wb16[:, B_P216 + DIM : B_P216 + 16] = 2.0

    wb32 = np.zeros((128, 4), np.float32)
    wb32[:, 0] = np.asarray(inputs["b0"], np.float32)
    wb32[:, 1] = np.asarray(inputs["b1"], np.float32)
    wb32[:, 2] = c0[:, 0]
    wb32[:, 3] = -W2.reshape(H)

    shared = {"WB16": wb16.astype(np.float16), "WB32": wb32}

    # batch permutation: device column 128J+p holds X row 64p+J so the
    # output DMA is 2KB-contiguous per partition.
    b = np.arange(BC)
    perm = (BC // 128) * (b % 128) + b // 128

    in_maps = []
    for i in range(NCORES):
        xp = X[i * BC : (i + 1) * BC][perm]
        m = {"XT16": np.ascontiguousarray(xp.T).astype(np.float16)}
        m.update(shared)
        in_maps.append(m)

    res = run_bass_kernel_spmd(
        nc, in_maps, core_ids=list(range(NCORES)), trace=trace
    )
    LAST_RESULTS = res
    out_full = np.concatenate([res.results[i]["out"] for i in range(NCORES)], axis=0)
    return out_full.astype(np.float32)
